# revision 1
# baseline (speedup 1.0000x reference)
"""Trainium2 Bass kernel for masked multi-adapter LoRA (moe_routing).

Computes out = result + ((x @ A_cat) * onehot_mask) @ B_cat  where
A_cat [H, 128] stacks the 8 adapters' shrink matrices along the rank dim and
B_cat [128, O] stacks the expand matrices.  Since each token's one-hot mask
zeroes every rank column except its own adapter's 16, this equals the
reference's per-adapter masked update loop exactly (masked terms add 0.0).

Sharding: data-parallel over tokens, T=8192 -> 1024 tokens per core x 8 cores.
Each core runs an identical program (SPMD) on its token shard with the small
adapter stacks replicated.

Per-core pipeline (fp32 end to end):
  - x tiles [128, 4096] are DMA'd in natural token-major layout, transposed
    128x128-blockwise on the PE (exact for fp32) into xT tiles [H-part, tok].
  - shrink: VT[128rc, 512tok] accumulated in PSUM over 32 H-chunks with
    A_cat chunks as the stationary operand.
  - mask: one DVE multiply against the host-built one-hot mask (transposed
    layout [128rc, tok]) drains PSUM -> SBUF.
  - expand: VmT token-slices become the stationary operand; B_cat streams.
  - result tiles are added on DVE (in place) and stored back.
"""

import numpy as np
from contextlib import ExitStack

import concourse.bass as bass
import concourse.mybir as mybir
import concourse.tile as tile
from concourse import bacc
from concourse.bass_utils import run_bass_kernel_spmd
from concourse.masks import make_identity

# problem shape (hardcoded per harness contract)
T, H, R, O, NA = 8192, 4096, 16, 4096, 8
NCORES = 8
TS = T // NCORES            # tokens per core = 1024
P = 128
RC = NA * R                 # concatenated rank dim = 128
KC = H // P                 # 32 H-chunks
SB = 512                    # superblock tokens (PSUM bank free-dim)
NSB = TS // SB              # 2 superblocks per core
G = SB // P                 # 4 token tiles per superblock
NJ = O // 512               # 8 expand column chunks

F32 = mybir.dt.float32

_BUILT = {}


def _emit(tc, x, res, a_cat, b_cat, maskT, out, repeats=1, use_f32r=False):
    nc = tc.nc
    F32R = mybir.dt.float32r
    MMDT = F32R if use_f32r else F32
    ctx = ExitStack()
    with ctx:
        const = ctx.enter_context(tc.tile_pool(name="const", bufs=1))
        xpool = ctx.enter_context(tc.tile_pool(name="xpool", bufs=5))
        xtpool = ctx.enter_context(tc.tile_pool(name="xtpool", bufs=6))
        vpool = ctx.enter_context(tc.tile_pool(name="vpool", bufs=2))
        rpool = ctx.enter_context(tc.tile_pool(name="rpool", bufs=2))
        tp_ps_pool = ctx.enter_context(tc.tile_pool(name="tp_ps", bufs=3, space="PSUM"))
        vt_ps_pool = ctx.enter_context(tc.tile_pool(name="vt_ps", bufs=2, space="PSUM"))
        u_ps_pool = ctx.enter_context(tc.tile_pool(name="u_ps", bufs=3, space="PSUM"))

        # 3D views: token tiles of 128
        x3 = x.rearrange("(t p) h -> t p h", p=P)
        res3 = res.rearrange("(t p) o -> t p o", p=P)
        out3 = out.rearrange("(t p) o -> t p o", p=P)
        a3 = a_cat.rearrange("(ko p) m -> p ko m", p=P)

        # resident tensors
        a_sb = const.tile([P, KC, P], F32, name="a_sb")
        nc.sync.dma_start(a_sb[:], a3)
        b_sb = const.tile([P, O], F32, name="b_sb")
        nc.sync.dma_start(b_sb[:], b_cat)
        if use_f32r:
            a_r = const.tile([P, KC, P], F32R, name="a_r")
            nc.vector.tensor_copy(a_r[:], a_sb[:])
            b_r = const.tile([P, O], F32R, name="b_r")
            nc.vector.tensor_copy(b_r[:], b_sb[:])
        else:
            a_r, b_r = a_sb, b_sb
        m_sb = const.tile([P, TS], F32, name="m_sb")
        nc.sync.dma_start(m_sb[:], maskT)
        ident = const.tile([P, P], F32, name="ident")
        make_identity(nc, ident[:])

        for rep in range(repeats):
            for sb in range(NSB):
                # load the 4 x token-tiles of this superblock
                xg = []
                for g in range(G):
                    xt = xpool.tile([P, H], F32, name=f"xg_{rep}_{sb}_{g}", tag="xg")
                    nc.sync.dma_start(xt[:], x3[sb * G + g])
                    xg.append(xt)

                # shrink: VT[rc, tok] accumulated over 32 H-chunks
                vt_ps = vt_ps_pool.tile([P, SB], F32, name=f"vt_{rep}_{sb}", tag="vt")
                for k in range(KC):
                    tp_ps = tp_ps_pool.tile([P, SB], F32, name=f"tp_{rep}_{sb}_{k}", tag="tp")
                    for g in range(G):
                        nc.tensor.transpose(
                            tp_ps[:, g * P:(g + 1) * P],
                            xg[g][:, k * P:(k + 1) * P],
                            ident[:],
                        )
                    xT = xtpool.tile([P, SB], MMDT, name=f"xT_{rep}_{sb}_{k}", tag="xT")
                    if use_f32r:
                        nc.vector.tensor_copy(xT[:], tp_ps[:])
                    else:
                        nc.scalar.copy(xT[:], tp_ps[:])
                    nc.tensor.matmul(
                        vt_ps[:], a_r[:, k], xT[:],
                        start=(k == 0), stop=(k == KC - 1),
                    )

                # mask (drains PSUM -> SBUF)
                vmT = vpool.tile([P, SB], MMDT, name=f"vmT_{rep}_{sb}", tag="vmT")
                nc.vector.tensor_tensor(
                    vmT[:], vt_ps[:], m_sb[:, sb * SB:(sb + 1) * SB],
                    mybir.AluOpType.mult,
                )

                # expand + result add + store, one token tile at a time
                for g in range(G):
                    r_sb = rpool.tile([P, O], F32, name=f"r_{rep}_{sb}_{g}", tag="r")
                    nc.sync.dma_start(r_sb[:], res3[sb * G + g])
                    for j in range(NJ):
                        u_ps = u_ps_pool.tile([P, 512], F32, name=f"u_{rep}_{sb}_{g}_{j}", tag="u")
                        nc.tensor.matmul(
                            u_ps[:], vmT[:, g * P:(g + 1) * P],
                            b_r[:, j * 512:(j + 1) * 512],
                            start=True, stop=True,
                        )
                        nc.vector.tensor_tensor(
                            r_sb[:, j * 512:(j + 1) * 512], u_ps[:],
                            r_sb[:, j * 512:(j + 1) * 512],
                            mybir.AluOpType.add,
                        )
                    nc.sync.dma_start(out3[sb * G + g], r_sb[:])


def build(repeats=1, use_f32r=False):
    """Build + compile the per-core Bass program (shared by all 8 cores)."""
    nc = bacc.Bacc("TRN2", target_bir_lowering=False, debug=False,
                   num_devices=NCORES)
    x = nc.dram_tensor("x", [TS, H], F32, kind="ExternalInput").ap()
    res = nc.dram_tensor("res", [TS, O], F32, kind="ExternalInput").ap()
    a_cat = nc.dram_tensor("a_cat", [H, RC], F32, kind="ExternalInput").ap()
    b_cat = nc.dram_tensor("b_cat", [RC, O], F32, kind="ExternalInput").ap()
    maskT = nc.dram_tensor("maskT", [RC, TS], F32, kind="ExternalInput").ap()
    out = nc.dram_tensor("out", [TS, O], F32, kind="ExternalOutput").ap()

    with tile.TileContext(nc) as tc:
        _emit(tc, x, res, a_cat, b_cat, maskT, out, repeats=repeats,
              use_f32r=use_f32r)
    nc.compile()
    return nc


def make_in_maps(result, x, lora_a, lora_b, adapter_indices):
    result = np.asarray(result, dtype=np.float32)
    x = np.asarray(x, dtype=np.float32)
    lora_a = np.asarray(lora_a, dtype=np.float32)
    lora_b = np.asarray(lora_b, dtype=np.float32)
    idx = np.asarray(adapter_indices, dtype=np.int32)

    a_cat = np.ascontiguousarray(lora_a.transpose(1, 0, 2).reshape(H, RC))
    b_cat = np.ascontiguousarray(lora_b.reshape(RC, O))
    c16 = (np.arange(RC) // R).astype(np.int32)

    in_maps = []
    for c in range(NCORES):
        sl = slice(c * TS, (c + 1) * TS)
        mT = (idx[sl][None, :] == c16[:, None]).astype(np.float32)
        in_maps.append({
            "x": np.ascontiguousarray(x[sl]),
            "res": np.ascontiguousarray(result[sl]),
            "a_cat": a_cat,
            "b_cat": b_cat,
            "maskT": np.ascontiguousarray(mT),
        })
    return in_maps


def kernel(result, x, lora_a, lora_b, adapter_indices):
    # f32r matmul operands: ~2.4e-4 max rel err vs the fp32 reference while
    # hitting the HBM roofline (pure-fp32 PE runs at 1/4 rate and lands
    # ~25% slower; flip use_f32r=False for 4e-7 accuracy if ever needed).
    in_maps = make_in_maps(result, x, lora_a, lora_b, adapter_indices)
    if "nc" not in _BUILT:
        _BUILT["nc"] = build(use_f32r=True)
    res = run_bass_kernel_spmd(_BUILT["nc"], in_maps, core_ids=list(range(NCORES)))
    return np.concatenate(
        [res.results[c]["out"] for c in range(NCORES)], axis=0
    ).astype(np.float32)


if __name__ == "__main__":
    rng = np.random.default_rng(0)
    inputs = {
        "result": rng.standard_normal((T, O), dtype=np.float32),
        "x": rng.standard_normal((T, H), dtype=np.float32),
        "lora_a": rng.standard_normal((NA, H, R), dtype=np.float32),
        "lora_b": rng.standard_normal((NA, R, O), dtype=np.float32),
        "adapter_indices": rng.integers(0, NA, size=(T,), dtype=np.int32),
    }
    out = kernel(**inputs)
    print("kernel output:", out.shape, out.dtype)

